# revision 13
# baseline (speedup 1.0000x reference)
"""StyleGAN2 modulated conv_transpose (stride=1, pad=1) for Trainium2.

Strategy (data-parallel over batch, 2 samples per core on 8 cores):
  conv_transpose2d(x, w_mod) with per-sample modulated+demodulated weights
  factors exactly as
      out_b[o] = (GAIN/d_b[o]) * conv2d(s_b (.) x_b, W*HE)[o] + GAIN*bias[o]
      d_b[o]   = sqrt(HE^2 * sum_i s_b[i]^2 * R[i,o] + eps),  R = sum_taps W^2
  so all samples share one weight tensor:
    - DVE: scale input channels by style into a zero-padded (34x34) SBUF image
    - PE:  9 shifted-window matmuls x 4 k-tiles accumulate each (128 out x 512
           spatial) PSUM tile; demod norms via a tiny (N=2) PE matmul over R
    - ACT: copy-out fused with per-(sample,out) scale and bias
"""

from contextlib import ExitStack

import numpy as np

import concourse.bass as bass
from concourse import bacc
import concourse.mybir as mybir
import concourse.tile as tile
from concourse.bass_utils import run_bass_kernel_spmd

# matmul dtype mode: "f32" (exact, 4 cyc/row), "f32r" (fast fp32, 1 cyc/row),
# "bf16" (fast, ~2e-3 rel err)
MODE = "f32r"
TRACE = False
TRACE_KW = {}
LAST_RESULT = None

B, C, H, W, KK = 16, 512, 32, 32, 3
HW = H * W
NCORES, BPC = 8, B // 8
KT = C // 128  # k-tiles over in-channels
MT = C // 128  # m-tiles over out-channels
NT = 2         # spatial halves: N = 512 = 16 rows of 32
ROWS_N = H // NT
PADH = H + 2
GAIN = 1.4142135623730951
HE = GAIN / float(C * KK * KK) ** 0.5
EPS = 1e-8

F32 = mybir.dt.float32


def _build(mode):
    pad_dt = {"f32": F32, "f32r": mybir.dt.float32r, "bf16": mybir.dt.bfloat16}[mode]
    nc = bacc.Bacc("TRN2", target_bir_lowering=False)
    x_d = nc.declare_dram_parameter("x", [BPC, C, HW], F32, isOutput=False)
    wt_d = nc.declare_dram_parameter("wt", [KK * KK, C, C], F32, isOutput=False)
    st_d = nc.declare_dram_parameter("style", [BPC, C], F32, isOutput=False)
    bi_d = nc.declare_dram_parameter("bias", [C], F32, isOutput=False)
    out_d = nc.declare_dram_parameter("out", [BPC, C, HW], F32, isOutput=True)

    with tile.TileContext(nc) as tc, ExitStack() as ctx:
        singles = ctx.enter_context(tc.tile_pool(name="singles", bufs=1))
        stage = ctx.enter_context(tc.tile_pool(name="stage", bufs=3))
        wstage = ctx.enter_context(tc.tile_pool(name="wstage", bufs=2))
        tmps = ctx.enter_context(tc.tile_pool(name="tmps", bufs=3))
        osbp = ctx.enter_context(tc.tile_pool(name="osbp", bufs=4))
        cpsum = ctx.enter_context(tc.tile_pool(name="cpsum", bufs=6, space="PSUM"))
        dpsum = ctx.enter_context(tc.tile_pool(name="dpsum", bufs=1, space="PSUM"))

        # ---- small constants: style, style^2, GAIN*bias ----
        s_t = singles.tile([128, KT, BPC], F32, tag="s_t")
        for b in range(BPC):
            nc.gpsimd.dma_start(
                out=s_t[:, :, b], in_=st_d[b].rearrange("(k p) -> p k", p=128)
            )
        s2_t = singles.tile([128, KT, BPC], F32, tag="s2_t")
        nc.vector.tensor_mul(s2_t, s_t, s_t)
        gb_t = singles.tile([128, MT], F32, tag="gb_t")
        nc.gpsimd.dma_start(out=gb_t, in_=bi_d[:].rearrange("(m p) -> p m", p=128))
        nc.vector.tensor_scalar_mul(gb_t, gb_t, float(GAIN))

        # ---- weights: (p=in%128, tap, ktile, out) ----
        # stream each tap through an f32 stage; cast (for f32r/bf16) and
        # accumulate R = sum_taps W^2 while it is live
        w_mm = singles.tile([128, KK * KK, KT, C], pad_dt, tag="w_mm")
        R_t = singles.tile([128, KT, C], F32, tag="R_t")
        for t in range(KK * KK):
            if mode == "f32":
                ws = w_mm[:, t]
            else:
                ws = wstage.tile([128, KT, C], F32, tag="ws")
            nc.sync.dma_start(
                out=ws, in_=wt_d[t].rearrange("(k p) o -> p k o", p=128)
            )
            if mode != "f32":
                nc.vector.tensor_copy(out=w_mm[:, t], in_=ws)
            for k in range(KT):
                if t == 0:
                    nc.scalar.square(R_t[:, k], ws[:, k])
                else:
                    sq = tmps.tile([128, C], F32, tag="sq")
                    nc.scalar.square(sq, ws[:, k])
                    nc.vector.tensor_add(R_t[:, k], R_t[:, k], sq)

        # ---- inputs: style-scaled into zero-padded (34,34) images ----
        pads = {}
        for b in range(BPC):
            for k in range(KT):
                pt = singles.tile([128, PADH, PADH], pad_dt, tag=f"pad_{b}_{k}")
                xs = stage.tile([128, PADH, PADH], F32, tag="xs")
                nc.vector.memset(xs, 0.0)
                nc.sync.dma_start(
                    out=xs[:, 1 : H + 1, 1 : W + 1],
                    in_=x_d[b]
                    .rearrange("(k p) (h w) -> k p h w", p=128, h=H)[k],
                )
                nc.vector.tensor_scalar_mul(pt, xs, s_t[:, k, b : b + 1])
                pads[b, k] = pt

        d2p = dpsum.tile([128, MT, BPC], F32, tag="d2p")
        dinv = singles.tile([128, MT, BPC], F32, tag="dinv")

        # ---- conv: out[m,b,n] (128 out x 512 spatial) ----
        for m in range(MT):
            cps = {}
            for b in range(BPC):
                for n in range(NT):
                    cp = cpsum.tile([128, ROWS_N * W], F32, tag="cps")
                    cps[b, n] = cp
            for t in range(KK * KK):
                a, bw = divmod(t, 3)
                for k in range(KT):
                    lhsT = w_mm[:, t, k, m * 128 : (m + 1) * 128]
                    for b in range(BPC):
                        pt = pads[b, k]
                        for n in range(NT):
                            r0 = n * ROWS_N + (2 - a)
                            c0 = 2 - bw
                            rhs = pt[:, r0 : r0 + ROWS_N, c0 : c0 + W]
                            nc.tensor.matmul(
                                cps[b, n],
                                lhsT,
                                rhs,
                                start=(t == 0 and k == 0),
                                stop=(t == KK * KK - 1 and k == KT - 1),
                            )
            if m == 0:
                # demod norms: d2[o, b] = sum_i s2[i,b] * R[i,o]
                for m2 in range(MT):
                    for k in range(KT):
                        nc.tensor.matmul(
                            d2p[:, m2],
                            R_t[:, k, m2 * 128 : (m2 + 1) * 128],
                            s2_t[:, k],
                            start=(k == 0),
                            stop=(k == KT - 1),
                        )
                # dinv = GAIN*HE/sqrt(HE^2*d2 + EPS) = 1/sqrt(d2/GAIN^2 + EPS/(HE*GAIN)^2)
                dsq = singles.tile([128, MT, BPC], F32, tag="dsq")
                eps_t = singles.tile([128, 1], F32, tag="eps_t")
                nc.vector.memset(eps_t, float(EPS / (HE * HE * GAIN * GAIN)))
                nc.scalar.activation(
                    dsq,
                    d2p,
                    mybir.ActivationFunctionType.Sqrt,
                    bias=eps_t,
                    scale=float(1.0 / (GAIN * GAIN)),
                )
                nc.vector.reciprocal(dinv, dsq)
            for b in range(BPC):
                for n in range(NT):
                    osb = osbp.tile([128, ROWS_N * W], F32, tag="osb")
                    nc.scalar.activation(
                        osb,
                        cps[b, n],
                        mybir.ActivationFunctionType.Identity,
                        bias=gb_t[:, m : m + 1],
                        scale=dinv[:, m, b : b + 1],
                    )
                    nc.sync.dma_start(
                        out=out_d[b].rearrange("(mm p) s -> mm p s", p=128)[m][
                            :, n * ROWS_N * W : (n + 1) * ROWS_N * W
                        ],
                        in_=osb,
                    )
    nc.finalize()
    return nc


def kernel(inp, style, weight, bias):
    global LAST_RESULT
    inp = np.ascontiguousarray(np.asarray(inp, np.float32)).reshape(B, C, HW)
    w_t = np.ascontiguousarray(
        np.asarray(weight, np.float32).transpose(2, 3, 0, 1)
    ).reshape(KK * KK, C, C)
    style = np.ascontiguousarray(np.asarray(style, np.float32))
    bias = np.ascontiguousarray(np.asarray(bias, np.float32))

    nc = _build(MODE)
    in_maps = []
    for c in range(NCORES):
        sl = slice(c * BPC, (c + 1) * BPC)
        in_maps.append(
            {"x": inp[sl], "wt": w_t, "style": style[sl], "bias": bias}
        )
    res = run_bass_kernel_spmd(
        nc, in_maps, list(range(NCORES)), trace=TRACE, **TRACE_KW
    )
    LAST_RESULT = res
    out = np.concatenate([res.results[c]["out"] for c in range(NCORES)], axis=0)
    return out.reshape(B, C, H, W)


# revision 16
# speedup vs baseline: 1.1685x; 1.1685x over previous
"""StyleGAN2 modulated conv_transpose (stride=1, pad=1) for Trainium2.

Strategy (data-parallel over batch, 2 samples per core on 8 cores):
  conv_transpose2d(x, w_mod) with per-sample modulated+demodulated weights
  factors exactly as
      out_b[o] = (GAIN/d_b[o]) * conv2d(s_b (.) x_b, W*HE)[o] + GAIN*bias[o]
      d_b[o]   = sqrt(HE^2 * sum_i s_b[i]^2 * R[i,o] + eps),  R = sum_taps W^2
  so all samples share one weight tensor:
    - DVE: scale input channels by style (contiguous 32x32 images, no padding;
           conv boundary handled by shrunken matmul windows)
    - PE:  9 shifted-window matmuls x 4 k-tiles accumulate each (128 out x 512
           spatial) PSUM tile; demod norms via a tiny (N=2) PE matmul over R
    - ACT/DVE: copy-out fused with per-(sample,out) scale and bias
  Input DMAs are spread across the SP + ACT HWDGE queues and 4 SWDGE queues.
"""

from contextlib import ExitStack

import numpy as np

import concourse.bass as bass
from concourse import bacc
import concourse.mybir as mybir
import concourse.tile as tile
from concourse.bass_utils import run_bass_kernel_spmd

# matmul dtype mode: "f32" (exact, 4 cyc/row), "f32r" (fast fp32, 1 cyc/row),
# "bf16" (fast, ~2e-3 rel err)
MODE = "f32r"
TRACE = False
TRACE_KW = {}
LAST_RESULT = None

B, C, H, W, KK = 16, 512, 32, 32, 3
HW = H * W
NCORES, BPC = 8, B // 8
KT = C // 128  # k-tiles over in-channels
MT = C // 128  # m-tiles over out-channels
NT = 2         # spatial halves: N = 512 = 16 rows of 32
ROWS_N = H // NT
GAIN = 1.4142135623730951
HE = GAIN / float(C * KK * KK) ** 0.5
EPS = 1e-8

TAP_ORDER = [4, 0, 1, 2, 3, 5, 6, 7, 8]  # center tap first (full window)

F32 = mybir.dt.float32


def _build(mode):
    pad_dt = {"f32": F32, "f32r": mybir.dt.float32r, "bf16": mybir.dt.bfloat16}[mode]
    nc = bacc.Bacc("TRN2", target_bir_lowering=False, num_swdge_queues=4)
    x_d = nc.declare_dram_parameter("x", [BPC, C, HW], F32, isOutput=False)
    wt_d = nc.declare_dram_parameter("wt", [KK * KK, C, C], F32, isOutput=False)
    st_d = nc.declare_dram_parameter("style", [BPC, C], F32, isOutput=False)
    bi_d = nc.declare_dram_parameter("bias", [C], F32, isOutput=False)
    out_d = nc.declare_dram_parameter("out", [BPC, C, HW], F32, isOutput=True)

    with tile.TileContext(nc) as tc, ExitStack() as ctx:
        singles = ctx.enter_context(tc.tile_pool(name="singles", bufs=1))
        stage = ctx.enter_context(tc.tile_pool(name="stage", bufs=4))
        wstage = ctx.enter_context(tc.tile_pool(name="wstage", bufs=2))
        tmps = ctx.enter_context(tc.tile_pool(name="tmps", bufs=3))
        osbp = ctx.enter_context(tc.tile_pool(name="osbp", bufs=4))
        cpsum = ctx.enter_context(tc.tile_pool(name="cpsum", bufs=6, space="PSUM"))
        dpsum = ctx.enter_context(tc.tile_pool(name="dpsum", bufs=1, space="PSUM"))

        # ---- small constants: style, style^2, GAIN*bias ----
        s_t = singles.tile([128, KT, BPC], F32, tag="s_t")
        for b in range(BPC):
            nc.gpsimd.dma_start(
                out=s_t[:, :, b], in_=st_d[b].rearrange("(k p) -> p k", p=128)
            )
        s2_t = singles.tile([128, KT, BPC], F32, tag="s2_t")
        nc.vector.tensor_mul(s2_t, s_t, s_t)
        gb_t = singles.tile([128, MT], F32, tag="gb_t")
        nc.gpsimd.dma_start(out=gb_t, in_=bi_d[:].rearrange("(m p) -> p m", p=128))
        nc.vector.tensor_scalar_mul(gb_t, gb_t, float(GAIN))

        # ---- inputs: style-scaled (128, 32 rows, 34 cols) images with zero
        # columns 0/33 (conv col-padding); row padding via shrunken windows.
        # x DMAs spread over SP/ACT HWDGE + SWDGE queues.
        zc_t = singles.tile([128, H, 2], F32, tag="zc_t")
        nc.vector.memset(zc_t, 0.0)
        x_engines = [nc.sync, nc.scalar, nc.gpsimd, nc.gpsimd]
        pads = {}
        i = 0
        for k in range(KT):
            for b in range(BPC):
                xs = stage.tile([128, H, W], F32, tag="xs")
                x_engines[i % 4].dma_start(
                    out=xs,
                    in_=x_d[b].rearrange("(k p) (h w) -> k p h w", p=128, h=H)[k],
                )
                pt = singles.tile([128, H, W + 2], pad_dt, tag=f"pad_{b}_{k}")
                nc.vector.tensor_scalar_mul(
                    pt[:, :, 1 : W + 1], xs, s_t[:, k, b : b + 1]
                )
                # zero columns 0 and 33 in one strided copy
                border = bass.AP(
                    tensor=pt.tensor,
                    offset=pt.offset,
                    ap=[pt.ap[0], [W + 2, H], [W + 1, 2]],
                )
                nc.vector.tensor_copy(out=border, in_=zc_t)
                pads[b, k] = pt
                i += 1

        # ---- weights: stream per tap; cast + R = sum_taps W^2 while live ----
        w_engines = [nc.gpsimd, nc.sync, nc.scalar]
        w_mm = singles.tile([128, KK * KK, KT, C], pad_dt, tag="w_mm")
        R_t = singles.tile([128, KT, C], F32, tag="R_t")
        for ti, t in enumerate(TAP_ORDER):
            if mode == "f32":
                ws = w_mm[:, t]
            else:
                ws = wstage.tile([128, KT, C], F32, tag="ws")
            w_engines[ti % 3].dma_start(
                out=ws, in_=wt_d[t].rearrange("(k p) o -> p k o", p=128)
            )
            if mode != "f32":
                nc.vector.tensor_copy(out=w_mm[:, t], in_=ws)
            for k in range(KT):
                if ti == 0:
                    nc.scalar.square(R_t[:, k], ws[:, k])
                else:
                    sq = tmps.tile([128, C], F32, tag="sq")
                    nc.scalar.square(sq, ws[:, k])
                    nc.vector.tensor_add(R_t[:, k], R_t[:, k], sq)

        d2p = dpsum.tile([128, MT, BPC], F32, tag="d2p")
        dinv = singles.tile([128, MT, BPC], F32, tag="dinv")

        # ---- conv: out[m,b,n] = (128 out x 16 rows x 32 cols) PSUM tiles ----
        out_engines = [nc.sync, nc.scalar]
        oi = 0
        for m in range(MT):
            cps = {}
            for b in range(BPC):
                for n in range(NT):
                    cp = cpsum.tile([128, ROWS_N, W], F32, tag="cps")
                    cps[b, n] = cp
            for ti, t in enumerate(TAP_ORDER):
                a, bw = divmod(t, 3)
                h_lo_g, h_hi_g = max(0, a - 1), min(H, H - 1 + a)
                for k in range(KT):
                    lhsT = w_mm[:, t, k, m * 128 : (m + 1) * 128]
                    for b in range(BPC):
                        pt = pads[b, k]
                        for n in range(NT):
                            h_lo = max(n * ROWS_N, h_lo_g)
                            h_hi = min((n + 1) * ROWS_N, h_hi_g)
                            out_ap = cps[b, n][
                                :, h_lo - n * ROWS_N : h_hi - n * ROWS_N, :
                            ]
                            rhs = pt[
                                :,
                                h_lo + 1 - a : h_hi + 1 - a,
                                2 - bw : 2 - bw + W,
                            ]
                            nc.tensor.matmul(
                                out_ap,
                                lhsT,
                                rhs,
                                start=(ti == 0 and k == 0),
                                stop=(ti == KK * KK - 1 and k == KT - 1),
                            )
            if m == 0:
                # demod norms: d2[o, b] = sum_i s2[i,b] * R[i,o]
                for m2 in range(MT):
                    for k in range(KT):
                        nc.tensor.matmul(
                            d2p[:, m2],
                            R_t[:, k, m2 * 128 : (m2 + 1) * 128],
                            s2_t[:, k],
                            start=(k == 0),
                            stop=(k == KT - 1),
                        )
                # dinv = GAIN*HE/sqrt(HE^2*d2+EPS) = 1/sqrt(d2/G^2 + EPS/(HE*G)^2)
                dsq = singles.tile([128, MT, BPC], F32, tag="dsq")
                eps_t = singles.tile([128, 1], F32, tag="eps_t")
                nc.vector.memset(eps_t, float(EPS / (HE * HE * GAIN * GAIN)))
                nc.scalar.activation(
                    dsq,
                    d2p,
                    mybir.ActivationFunctionType.Sqrt,
                    bias=eps_t,
                    scale=float(1.0 / (GAIN * GAIN)),
                )
                nc.vector.reciprocal(dinv, dsq)
            for b in range(BPC):
                for n in range(NT):
                    osb = osbp.tile([128, ROWS_N * W], F32, tag="osb")
                    cp_flat = cps[b, n].rearrange("p r w -> p (r w)")
                    if (b + n) % 2 == 0:
                        nc.scalar.activation(
                            osb,
                            cp_flat,
                            mybir.ActivationFunctionType.Identity,
                            bias=gb_t[:, m : m + 1],
                            scale=dinv[:, m, b : b + 1],
                        )
                    else:
                        nc.vector.tensor_scalar(
                            osb,
                            cp_flat,
                            dinv[:, m, b : b + 1],
                            gb_t[:, m : m + 1],
                            op0=mybir.AluOpType.mult,
                            op1=mybir.AluOpType.add,
                        )
                    out_engines[oi % 2].dma_start(
                        out=out_d[b].rearrange("(mm p) s -> mm p s", p=128)[m][
                            :, n * ROWS_N * W : (n + 1) * ROWS_N * W
                        ],
                        in_=osb,
                    )
                    oi += 1
    nc.finalize()
    return nc


def kernel(inp, style, weight, bias):
    global LAST_RESULT
    inp = np.ascontiguousarray(np.asarray(inp, np.float32)).reshape(B, C, HW)
    w_t = np.ascontiguousarray(
        np.asarray(weight, np.float32).transpose(2, 3, 0, 1)
    ).reshape(KK * KK, C, C)
    style = np.ascontiguousarray(np.asarray(style, np.float32))
    bias = np.ascontiguousarray(np.asarray(bias, np.float32))

    nc = _build(MODE)
    in_maps = []
    for c in range(NCORES):
        sl = slice(c * BPC, (c + 1) * BPC)
        in_maps.append(
            {"x": inp[sl], "wt": w_t, "style": style[sl], "bias": bias}
        )
    res = run_bass_kernel_spmd(
        nc, in_maps, list(range(NCORES)), trace=TRACE, **TRACE_KW
    )
    LAST_RESULT = res
    out = np.concatenate([res.results[c]["out"] for c in range(NCORES)], axis=0)
    return out.reshape(B, C, H, W)
